# revision 7
# baseline (speedup 1.0000x reference)
"""Trainium2 Bass kernel for an 8-expert top-2 MoE (SwiGLU experts).

Problem shapes: T=256 tokens, H=1024 hidden, I=4096 intermediate,
E=8 experts, top_k=2, fp32 reference.

Strategy (expert parallel over 8 NeuronCores, bf16 weights, token gather):
  - Core c holds expert c's weights in bf16 (w1s[c], w2s[c], w3s[c]):
    24 MiB, streamed from HBM (~56 us at the measured ~450 GB/s) -- the
    memory roofline. bf16 weight quantization adds ~0.3% relative error,
    far inside the 2e-2 budget.
  - The router (gate matmul + softmax + top-2 + renormalize) is replicated
    on every core in exact fp32 (the top-2 margin can be as small as 1e-4,
    so bf16/fp32r routing would mis-route tokens); the gate matrix is fed
    with its columns rotated per-core so column 0 is the core's own expert.
  - Each core compacts the tokens routed to its expert into C=128 capacity
    slots (expected load 64 +- 7, so C=128 is a >9-sigma bound):
      pos[t]  = exclusive cumsum of the selection mask (matmul with a
                strict-lower-triangular ones matrix),
      S[t,c]  = (pos[t] == c) * mask[t]   (one-hot slot map, DVE compares),
      xg      = x^T @ S                   (gather by matmul, bf16).
  - SwiGLU MLP on the slots with the gathered activations STATIONARY and
    the w1/w3 group columns moving (8 matmuls x 512 moving columns per
    1 MiB weight group instead of 32 x 128): minimizes PE instruction
    count + weight-load rows so PE stays under the DMA roofline. The
    slot-major SwiGLU product is PE-transposed back to i-major for the
    W2 accumulation chain.
  - The expert output is scaled by the per-slot combine weight (computed
    exactly in fp32 as S^T @ comb), scattered back to token order with
    S^T (a PE transpose of S), and combined across cores by a bf16
    ReduceScatter; the host concatenates the 8 token shards.
"""

import sys

if "/opt/trn_rl_repo" not in sys.path:
    sys.path.insert(0, "/opt/trn_rl_repo")

import numpy as np
import ml_dtypes

import concourse.bacc as bacc
import concourse.mybir as mybir
import concourse.tile as tile
from concourse.bass import ds as bass_ds, ts
from concourse.bass_utils import run_bass_kernel_spmd

T, H, I, E = 256, 1024, 4096, 8
N_CORES = 8
C = 128  # expert capacity (token slots per core)
HK = H // 128  # 8 h-chunks (contraction for w1/w3)
MK = I // 128  # 32 i-chunks (partition chunks of the intermediate)
GROUPS = 8  # w1/w3 weight-staging groups along I
MPG = MK // GROUPS  # 4 i-chunks per group
IG = I // GROUPS  # 512 intermediate columns per group
# w2 staging (i-chunks per stage): small first stage so the first w1/w3
# group isn't queued behind a full w2 stage; small last stages so the final
# exposed W2 chain after the last DMA byte is short.
W2_STAGES = (1, 3, 4, 4, 4, 4, 4, 4, 2, 2)
W2_START = (0, 1, 4, 8, 12, 16, 20, 24, 28, 30)
W2_STAGE_OF = sum(([s] * n for s, n in enumerate(W2_STAGES)), [])
TK = T // 128  # 2 token chunks
NH = H // 512  # 2 psum column-halves of the output's H axis

F32 = mybir.dt.float32
BF16 = mybir.dt.bfloat16
AF = mybir.ActivationFunctionType
ALU = mybir.AluOpType
AX = mybir.AxisListType
BF16NP = ml_dtypes.bfloat16


def build_nc(
    iters: int = 1,
    n_cores: int = N_CORES,
    with_collective: bool = True,
    silu_native: bool = True,
    debug_comb: bool = False,
    combine: str = "rs",
):
    """Build the SPMD program. `iters` repeats the whole compute body (for
    steady-state timing); the collective + output store run once at the end.
    `silu_native=False` lowers silu as sigmoid+mul (CoreSim has no Silu).
    `combine`: "rs" = bf16 ReduceScatter (output is this core's [T/8, H]
    token shard; host concatenates), "ar" = bf16 AllReduce (full output)."""
    nc = bacc.Bacc("TRN2", target_bir_lowering=False, debug=False, num_devices=n_cores)

    # Host-prepped inputs (see make_in_maps for the exact layouts).
    xT = nc.dram_tensor("xT", [128, HK, T], F32, kind="ExternalInput")
    xn = nc.dram_tensor("xn", [128, TK, H], BF16, kind="ExternalInput")
    gate = nc.dram_tensor("gate", [128, HK, E], F32, kind="ExternalInput")
    cum = nc.dram_tensor("cum", [128, 2 * 128], F32, kind="ExternalInput")
    iota = nc.dram_tensor("iota", [128, C], F32, kind="ExternalInput")
    ident = nc.dram_tensor("ident", [128, 128], BF16, kind="ExternalInput")
    w1 = nc.dram_tensor("w1", [128, GROUPS, HK * IG], BF16, kind="ExternalInput")
    w3 = nc.dram_tensor("w3", [128, GROUPS, HK * IG], BF16, kind="ExternalInput")
    w2 = nc.dram_tensor("w2", [128, MK, H], BF16, kind="ExternalInput")

    TS = T // n_cores  # output token-shard rows under ReduceScatter
    if combine == "rs" and with_collective:
        out = nc.dram_tensor("out", [TS, H], BF16, kind="ExternalOutput")
    else:
        out = nc.dram_tensor("out", [T, H], BF16, kind="ExternalOutput")
    if debug_comb:
        combdbg = nc.dram_tensor("combdbg", [T, 1], F32, kind="ExternalOutput")
        posdbg = nc.dram_tensor("posdbg", [T, 1], F32, kind="ExternalOutput")

    xT_v = xT.ap()
    xn_v = xn.ap()
    gate_v = gate.ap()
    w1_v = w1.ap()
    w3_v = w3.ap()
    w2_v = w2.ap()

    with tile.TileContext(nc) as tc:
        with (
            tc.tile_pool(name="zpool", bufs=2) as zpool,
            tc.tile_pool(name="w1p", bufs=3) as w1p,
            tc.tile_pool(name="w3p", bufs=3) as w3p,
            tc.tile_pool(name="w2p", bufs=2) as w2p,
            tc.tile_pool(name="hpool", bufs=4) as hpool,
            tc.tile_pool(name="hmtp", bufs=8) as hmtp,
            tc.tile_pool(name="small", bufs=2) as small,
            tc.tile_pool(name="gpool", bufs=2) as gpool,
            tc.tile_pool(name="outsb", bufs=2) as outsb,
            tc.tile_pool(name="ps_a", bufs=1, space="PSUM") as ps_a,
            tc.tile_pool(name="ps_h1", bufs=2, space="PSUM") as ps_h1,
            tc.tile_pool(name="ps_h3", bufs=1, space="PSUM") as ps_h3,
            tc.tile_pool(name="ps_t", bufs=2, space="PSUM") as ps_t,
            tc.tile_pool(name="ps_y", bufs=1, space="PSUM") as ps_y,
            tc.tile_pool(name="dram", bufs=1, space="DRAM") as dram,
        ):
            partial = dram.tile([T, H], BF16)  # collective input bounce
            if combine == "rs":
                reduced = dram.tile([TS, H], BF16)
            else:
                reduced = dram.tile([T, H], BF16)

            def body(_iv=None):
                # ---- activations + router/gather constants (fresh each iter)
                z = zpool.tile([128, HK, T], F32, tag="z")
                xn_sb = zpool.tile([128, TK, H], BF16, tag="xn")
                g_sb = zpool.tile([128, HK, E], F32, tag="g")
                cum_sb = zpool.tile([128, 2 * 128], F32, tag="cum")
                iota_sb = zpool.tile([128, C], F32, tag="iota")
                id_sb = zpool.tile([128, 128], BF16, tag="id")
                nc.gpsimd.dma_start(z[:], xT_v)
                nc.gpsimd.dma_start(xn_sb[:], xn_v)
                nc.gpsimd.dma_start(g_sb[:], gate_v)
                nc.gpsimd.dma_start(cum_sb[:], cum.ap())
                nc.gpsimd.dma_start(iota_sb[:], iota.ap())
                nc.gpsimd.dma_start(id_sb[:], ident.ap())

                # ---- router: logits -> softmax -> top-2 renormalized weight
                # for THIS core's expert (gate column 0). cb[t] in [128,1] is
                # the combine weight, 0 when the token skips this expert;
                # sel[t] is the 0/1 selection mask.
                combs, masks = [], []
                for t in range(TK):
                    ps_r = ps_a.tile([128, E], F32, tag="ra")
                    for hk in range(HK):
                        nc.tensor.matmul(
                            ps_r[:],
                            z[:, hk, ts(t, 128)],
                            g_sb[:, hk, :],
                            start=(hk == 0),
                            stop=(hk == HK - 1),
                        )
                    neg_mx = small.tile([128, 1], F32, tag="neg_mx")
                    nc.vector.tensor_reduce(
                        neg_mx[:], ps_r[:], AX.X, ALU.max, negate=True
                    )
                    ex = small.tile([128, E], F32, tag="ex")
                    nc.scalar.activation(ex[:], ps_r[:], AF.Exp, bias=neg_mx[:])
                    ssum = small.tile([128, 1], F32, tag="ssum")
                    nc.vector.tensor_reduce(ssum[:], ex[:], AX.X, ALU.add)
                    srec = small.tile([128, 1], F32, tag="srec")
                    nc.vector.reciprocal(srec[:], ssum[:])
                    p = small.tile([128, E], F32, tag="p")
                    nc.vector.tensor_scalar_mul(p[:], ex[:], srec[:])
                    m1 = small.tile([128, 1], F32, tag="m1")
                    nc.vector.tensor_reduce(m1[:], p[:], AX.X, ALU.max)
                    # knock out the top-1 entry; the max of the rest is top-2
                    pm = small.tile([128, E], F32, tag="pm")
                    nc.vector.tensor_single_scalar(pm[:], p[:], m1[:], ALU.is_equal)
                    p2 = small.tile([128, E], F32, tag="p2")
                    nc.vector.scalar_tensor_tensor(
                        p2[:], pm[:], -2.0, p[:], ALU.mult, ALU.add
                    )
                    m2 = small.tile([128, 1], F32, tag="m2")
                    nc.vector.tensor_reduce(m2[:], p2[:], AX.X, ALU.max)
                    denom = small.tile([128, 1], F32, tag="denom")
                    nc.vector.tensor_add(denom[:], m1[:], m2[:])
                    drec = small.tile([128, 1], F32, tag="drec")
                    nc.vector.reciprocal(drec[:], denom[:])
                    sel = small.tile([128, 1], F32, tag="sel")
                    nc.vector.tensor_single_scalar(
                        sel[:], p[:, 0:1], m2[:], ALU.is_ge
                    )
                    wn = small.tile([128, 1], F32, tag="wn")
                    nc.vector.tensor_scalar_mul(wn[:], p[:, 0:1], drec[:])
                    cb = small.tile([128, 1], F32, tag="cb")
                    nc.vector.tensor_mul(cb[:], wn[:], sel[:])
                    combs.append(cb)
                    masks.append(sel)
                    if debug_comb:
                        nc.sync.dma_start(combdbg[ts(t, 128), :], cb[:])

                # ---- token compaction: exclusive cumsum of the mask gives
                # each selected token its capacity slot; S (and its PE
                # transpose St) are the one-hot gather/scatter matrices.
                s_bf, s_f32 = [], []
                st_sb = gpool.tile([128, TK, 128], BF16, tag="st")
                for t in range(TK):
                    pos_ps = ps_a.tile([128, 1], F32, tag="ra")
                    if t == 0:
                        nc.tensor.matmul(
                            pos_ps[:], cum_sb[:, 0:128], masks[0][:],
                            start=True, stop=True,
                        )
                    else:
                        nc.tensor.matmul(
                            pos_ps[:], cum_sb[:, 128:256], masks[0][:],
                            start=True, stop=False,
                        )
                        nc.tensor.matmul(
                            pos_ps[:], cum_sb[:, 0:128], masks[1][:],
                            start=False, stop=True,
                        )
                    pos_sb = small.tile([128, 1], F32, tag="pos")
                    nc.scalar.copy(pos_sb[:], pos_ps[:])
                    if debug_comb:
                        nc.sync.dma_start(posdbg[ts(t, 128), :], pos_sb[:])
                    s_eq = gpool.tile([128, C], F32, tag="s_eq")
                    nc.vector.tensor_single_scalar(
                        s_eq[:], iota_sb[:], pos_sb[:], ALU.is_equal
                    )
                    s_m = gpool.tile([128, C], F32, tag="s_m")
                    nc.vector.tensor_scalar_mul(s_m[:], s_eq[:], masks[t][:])
                    s_b = gpool.tile([128, C], BF16, tag="s_b")
                    nc.vector.tensor_copy(s_b[:], s_m[:])
                    s_f32.append(s_m)
                    s_bf.append(s_b)
                    # St[c, t] = S[t, c] via PE transpose against identity
                    tr_ps = ps_t.tile([128, 128], BF16, tag="tr")
                    nc.tensor.transpose(tr_ps[:], s_b[:], id_sb[:])
                    nc.scalar.copy(st_sb[:, t, :], tr_ps[:])

                # per-slot combine weight, exact fp32: comb_g = S^T @ comb
                cg_ps = ps_a.tile([128, 1], F32, tag="ra")
                for t in range(TK):
                    nc.tensor.matmul(
                        cg_ps[:], s_f32[t][:], combs[t][:],
                        start=(t == 0), stop=(t == TK - 1),
                    )
                cg_sb = small.tile([128, 1], F32, tag="cg")
                nc.scalar.copy(cg_sb[:], cg_ps[:])

                # gathered activations, transposed layout: xg[h, c]
                z_g = zpool.tile([128, HK, C], BF16, tag="zg")
                for hk in range(HK):
                    xg_ps = ps_h3.tile([128, C], F32, tag="h3")
                    for t in range(TK):
                        nc.tensor.matmul(
                            xg_ps[:],
                            xn_sb[:, t, ts(hk, 128)],
                            s_bf[t][:],
                            start=(t == 0),
                            stop=(t == TK - 1),
                        )
                    nc.scalar.copy(z_g[:, hk, :], xg_ps[:])

                # ---- expert MLP on the C slots, grouped weight streaming.
                # z_g is the STATIONARY operand and the w1/w3 group columns
                # the moving one: 8 matmuls of 512 moving columns per group
                # instead of 32 of 128, slashing PE instruction count and
                # weight-load rows. The SwiGLU product comes out slot-major
                # [c, i]; PE-transpose it back to i-major 128-blocks for the
                # W2 accumulation chain.
                yg_ps = ps_y.tile([128, H], F32, tag="yg")
                hm_ci = [None] * GROUPS
                w2_sbs = {}

                def stage_w2(m):
                    s = W2_STAGE_OF[m]
                    if m != W2_START[s]:
                        return
                    nch = W2_STAGES[s]
                    w2_sbs[s] = w2p.tile(
                        [128, nch, H], BF16, tag="w2", name=f"w2sb{s}"
                    )
                    nc.sync.dma_start(
                        w2_sbs[s][:], w2_v[:, bass_ds(W2_START[s], nch), :]
                    )

                def tr_and_w2(g):
                    # transpose group g's SwiGLU product to i-major and run
                    # its W2 accumulation chains
                    for j in range(MPG):
                        m = g * MPG + j
                        tr_ps = ps_t.tile([128, 128], BF16, tag="tr")
                        nc.tensor.transpose(
                            tr_ps[:], hm_ci[g][:, ts(j, 128)], id_sb[:]
                        )
                        hmt = hmtp.tile([128, 128], BF16, tag="hmt")
                        nc.scalar.copy(hmt[:], tr_ps[:])
                        s = W2_STAGE_OF[m]
                        off = m - W2_START[s]
                        for n in range(NH):
                            nc.tensor.matmul(
                                yg_ps[:, ts(n, 512)],
                                hmt[:],
                                w2_sbs[s][:, off, ts(n, 512)],
                                start=(m == 0),
                                stop=(m == MK - 1),
                            )

                for g in range(GROUPS):
                    # first W2 stage goes ahead of w1/w3 in the DMA FIFO so
                    # the first W2 matmul chain never head-of-line-blocks PE
                    for j in range(MPG):
                        stage_w2(g * MPG + j)
                    w1_sb = w1p.tile([128, HK, IG], BF16, tag="w1")
                    w3_sb = w3p.tile([128, HK, IG], BF16, tag="w3")
                    nc.sync.dma_start(w1_sb[:], w1_v[:, g, :])
                    nc.sync.dma_start(w3_sb[:], w3_v[:, g, :])
                    h1m = ps_h1.tile([128, IG], F32, tag="h1")
                    h3m = ps_h3.tile([128, IG], F32, tag="h3")
                    for hk in range(HK):
                        nc.tensor.matmul(
                            h1m[:],
                            z_g[:, hk, :],
                            w1_sb[:, hk, :],
                            start=(hk == 0),
                            stop=(hk == HK - 1),
                        )
                    for hk in range(HK):
                        nc.tensor.matmul(
                            h3m[:],
                            z_g[:, hk, :],
                            w3_sb[:, hk, :],
                            start=(hk == 0),
                            stop=(hk == HK - 1),
                        )
                    h1s = hpool.tile([128, IG], F32, tag="h1s")
                    if silu_native:
                        nc.scalar.activation(h1s[:], h1m[:], AF.Silu)
                    else:
                        sg = hpool.tile([128, IG], F32, tag="sg")
                        nc.scalar.activation(sg[:], h1m[:], AF.Sigmoid)
                        nc.vector.tensor_mul(h1s[:], sg[:], h1m[:])
                    hm = hpool.tile([128, IG], BF16, tag="hmci")
                    nc.vector.tensor_mul(hm[:], h1s[:], h3m[:])
                    hm_ci[g] = hm
                    # transposes + W2 for the previous group: gives ACT/DVE a
                    # full group of slack to produce hm before PE needs it.
                    if g >= 1:
                        tr_and_w2(g - 1)
                tr_and_w2(GROUPS - 1)

                # ---- scale by combine weight, scatter to token order, store
                yg_sb = outsb.tile([128, H], BF16, tag="ygs")
                nc.vector.tensor_scalar_mul(yg_sb[:], yg_ps[:], cg_sb[:])
                for t in range(TK):
                    sc_ps = ps_y.tile([128, H], F32, tag="yg", name=f"sc{t}")
                    for n in range(NH):
                        nc.tensor.matmul(
                            sc_ps[:, ts(n, 512)], st_sb[:, t, :],
                            yg_sb[:, ts(n, 512)],
                            start=True, stop=True,
                        )
                    o_sb = outsb.tile([128, H], BF16, tag=f"o{t}")
                    nc.scalar.copy(o_sb[:], sc_ps[:])
                    nc.gpsimd.dma_start(partial[ts(t, 128), :], o_sb[:])

            if iters == 1:
                body()
            else:
                with tc.For_i(
                    0, iters, 1, hint_engines=(mybir.EngineType.PE,)
                ) as iv:
                    body(iv)

            if with_collective:
                nc.gpsimd.collective_compute(
                    "ReduceScatter" if combine == "rs" else "AllReduce",
                    ALU.add,
                    replica_groups=[list(range(n_cores))],
                    ins=[partial[:].opt()],
                    outs=[reduced[:].opt()],
                )
                nc.sync.dma_start(out[:], reduced[:])
            else:
                nc.sync.dma_start(out[:], partial[:])

    nc.compile()
    return nc


_CACHE = {}


def _built(key):
    if key not in _CACHE:
        _CACHE[key] = build_nc(*key)
    return _CACHE[key]


def _wlayout13(w):
    # [H, I] -> [hi=128, g, ho*ig] so each group DMA is one 8KB
    # descriptor per partition: out[hi, g, ho*IG+ig] = w[ho*128+hi, g*IG+ig]
    return np.ascontiguousarray(
        w.reshape(HK, 128, GROUPS, IG)
        .transpose(1, 2, 0, 3)
        .reshape(128, GROUPS, HK * IG)
        .astype(BF16NP)
    )


def _wlayout2(w):
    # [I, H] -> [ki=128, m, h]: out[ki, m, h] = w[m*128+ki, h]
    return np.ascontiguousarray(
        w.reshape(MK, 128, H).transpose(1, 0, 2).astype(BF16NP)
    )


def make_in_maps(hidden_states, gate_w, w1s, w2s, w3s, n_cores=N_CORES):
    x = np.asarray(hidden_states, dtype=np.float32)
    gate_w = np.asarray(gate_w, dtype=np.float32)
    w1s = np.asarray(w1s, dtype=np.float32)
    w2s = np.asarray(w2s, dtype=np.float32)
    w3s = np.asarray(w3s, dtype=np.float32)

    # x^T [H, T] -> [hi=128, ho, t] (fp32, feeds the exact router matmul)
    xT = np.ascontiguousarray(
        x.T.reshape(HK, 128, T).transpose(1, 0, 2).astype(np.float32)
    )
    # x [T, H] -> [ti=128, to, h] (bf16, stationary operand of the gather)
    xn = np.ascontiguousarray(
        x.reshape(TK, 128, H).transpose(1, 0, 2).astype(BF16NP)
    )
    # cumsum matrix: [:, 0:128][p, t] = (p < t), [:, 128:256] all-ones
    cum = np.zeros((128, 256), np.float32)
    cum[:, 0:128] = np.tril(np.ones((128, 128), np.float32), k=-1).T
    cum[:, 128:256] = 1.0
    iota = np.tile(np.arange(C, dtype=np.float32), (128, 1))
    ident = np.eye(128, dtype=BF16NP)

    in_maps = []
    for c in range(n_cores):
        g = np.roll(gate_w, -c, axis=1)  # column 0 = this core's expert
        in_maps.append(
            {
                "xT": xT,
                "xn": xn,
                "gate": np.ascontiguousarray(
                    g.reshape(HK, 128, E).transpose(1, 0, 2)
                ),
                "cum": cum,
                "iota": iota,
                "ident": ident,
                "w1": _wlayout13(w1s[c]),
                "w3": _wlayout13(w3s[c]),
                "w2": _wlayout2(w2s[c]),
            }
        )
    return in_maps


def kernel(hidden_states, gate_w, w1s, w2s, w3s):
    in_maps = make_in_maps(hidden_states, gate_w, w1s, w2s, w3s)
    nc = _built((1, N_CORES, True))
    res = run_bass_kernel_spmd(nc, in_maps, core_ids=list(range(N_CORES)))
    # ReduceScatter leaves token shard c on core c; concatenate the shards.
    return np.concatenate(
        [np.asarray(res.results[c]["out"]) for c in range(N_CORES)], axis=0
    ).astype(np.float32)


# revision 10
# speedup vs baseline: 1.0836x; 1.0836x over previous
"""Trainium2 Bass kernel for an 8-expert top-2 MoE (SwiGLU experts).

Problem shapes: T=256 tokens, H=1024 hidden, I=4096 intermediate,
E=8 experts, top_k=2, fp32 reference.

Strategy (expert parallel over 8 NeuronCores, bf16 weights, token gather):
  - Core c holds expert c's weights in bf16 (w1s[c], w2s[c], w3s[c]):
    24 MiB, streamed from HBM (~56 us at the measured ~450 GB/s) -- the
    memory roofline. bf16 weight quantization adds ~0.3% relative error,
    far inside the 2e-2 budget.
  - The router (gate matmul + softmax + top-2 + renormalize) is replicated
    on every core in exact fp32 (the top-2 margin can be as small as 1e-4,
    so bf16/fp32r routing would mis-route tokens); the gate matrix is fed
    with its columns rotated per-core so column 0 is the core's own expert.
  - Each core compacts the tokens routed to its expert into C=128 capacity
    slots (expected load 64 +- 7, so C=128 is a >9-sigma bound):
      pos[t]  = exclusive cumsum of the selection mask (matmul with a
                strict-lower-triangular ones matrix),
      S[t,c]  = (pos[t] == c) * mask[t]   (one-hot slot map, DVE compares),
      xg      = x^T @ S                   (gather by matmul, bf16).
  - SwiGLU MLP on the slots with the gathered activations STATIONARY and
    the w1/w3 group columns moving (8 matmuls x 512 moving columns per
    1 MiB weight group instead of 32 x 128): minimizes PE instruction
    count + weight-load rows so PE stays under the DMA roofline. The
    slot-major SwiGLU product is PE-transposed back to i-major for the
    W2 accumulation chain.
  - The expert output is scaled by the per-slot combine weight (computed
    exactly in fp32 as S^T @ comb), scattered back to token order with
    S^T (a PE transpose of S), and combined across cores by a bf16
    ReduceScatter; the host concatenates the 8 token shards.
"""

import sys

if "/opt/trn_rl_repo" not in sys.path:
    sys.path.insert(0, "/opt/trn_rl_repo")

import numpy as np
import ml_dtypes

import concourse.bacc as bacc
import concourse.mybir as mybir
import concourse.tile as tile
from concourse.bass import ds as bass_ds, ts
from concourse.bass_utils import run_bass_kernel_spmd

T, H, I, E = 256, 1024, 4096, 8
N_CORES = 8
C = 128  # expert capacity (token slots per core)
HK = H // 128  # 8 h-chunks (contraction for w1/w3)
MK = I // 128  # 32 i-chunks (partition chunks of the intermediate)
GROUPS = 8  # w1/w3 weight-staging groups along I
MPG = MK // GROUPS  # 4 i-chunks per group
IG = I // GROUPS  # 512 intermediate columns per group
# w2 staging (i-chunks per stage): small first stage so the first w1/w3
# group isn't queued behind a full w2 stage; small last stages so the final
# exposed W2 chain after the last DMA byte is short.
W2_STAGES = (1, 3, 4, 4, 4, 4, 4, 4, 2, 2)
W2_START = (0, 1, 4, 8, 12, 16, 20, 24, 28, 30)
W2_STAGE_OF = sum(([s] * n for s, n in enumerate(W2_STAGES)), [])
TK = T // 128  # 2 token chunks
NH = H // 512  # 2 psum column-halves of the output's H axis

F32 = mybir.dt.float32
BF16 = mybir.dt.bfloat16
AF = mybir.ActivationFunctionType
ALU = mybir.AluOpType
AX = mybir.AxisListType
BF16NP = ml_dtypes.bfloat16


def build_nc(
    iters: int = 1,
    n_cores: int = N_CORES,
    with_collective: bool = True,
    silu_native: bool = True,
    debug_comb: bool = False,
    combine: str = "rs",
    ablate: str = "",
):
    """Build the SPMD program. `iters` repeats the whole compute body (for
    steady-state timing); the collective + output store run once at the end.
    `silu_native=False` lowers silu as sigmoid+mul (CoreSim has no Silu).
    `combine`: "rs" = bf16 ReduceScatter (output is this core's [T/8, H]
    token shard; host concatenates), "ar" = bf16 AllReduce (full output)."""
    nc = bacc.Bacc("TRN2", target_bir_lowering=False, debug=False, num_devices=n_cores)

    # Host-prepped inputs (see make_in_maps for the exact layouts).
    xT = nc.dram_tensor("xT", [128, HK, T], F32, kind="ExternalInput")
    xn = nc.dram_tensor("xn", [128, TK, H], BF16, kind="ExternalInput")
    gate = nc.dram_tensor("gate", [128, HK, E], F32, kind="ExternalInput")
    cum = nc.dram_tensor("cum", [128, 2 * 128], F32, kind="ExternalInput")
    iota = nc.dram_tensor("iota", [128, C], F32, kind="ExternalInput")
    ident = nc.dram_tensor("ident", [128, 128], BF16, kind="ExternalInput")
    w1 = nc.dram_tensor("w1", [128, GROUPS, HK * IG], BF16, kind="ExternalInput")
    w3 = nc.dram_tensor("w3", [128, GROUPS, HK * IG], BF16, kind="ExternalInput")
    w2 = nc.dram_tensor("w2", [128, MK, H], BF16, kind="ExternalInput")

    TS = T // n_cores  # output token-shard rows under ReduceScatter
    if combine == "rs" and with_collective:
        out = nc.dram_tensor("out", [TS, H], BF16, kind="ExternalOutput")
    else:
        out = nc.dram_tensor("out", [T, H], BF16, kind="ExternalOutput")
    if debug_comb:
        combdbg = nc.dram_tensor("combdbg", [T, 1], F32, kind="ExternalOutput")
        posdbg = nc.dram_tensor("posdbg", [T, 1], F32, kind="ExternalOutput")

    xT_v = xT.ap()
    xn_v = xn.ap()
    gate_v = gate.ap()
    w1_v = w1.ap()
    w3_v = w3.ap()
    w2_v = w2.ap()

    with tile.TileContext(nc) as tc:
        with (
            tc.tile_pool(name="zpool", bufs=2) as zpool,
            tc.tile_pool(name="w1p", bufs=4) as w1p,
            tc.tile_pool(name="w3p", bufs=4) as w3p,
            tc.tile_pool(name="w2p", bufs=3) as w2p,
            tc.tile_pool(name="hpool", bufs=4) as hpool,
            tc.tile_pool(name="hmtp", bufs=8) as hmtp,
            tc.tile_pool(name="small", bufs=2) as small,
            tc.tile_pool(name="gpool", bufs=2) as gpool,
            tc.tile_pool(name="outsb", bufs=2) as outsb,
            tc.tile_pool(name="ps_a", bufs=1, space="PSUM") as ps_a,
            tc.tile_pool(name="ps_h1", bufs=2, space="PSUM") as ps_h1,
            tc.tile_pool(name="ps_h3", bufs=1, space="PSUM") as ps_h3,
            tc.tile_pool(name="ps_t", bufs=2, space="PSUM") as ps_t,
            tc.tile_pool(name="ps_y", bufs=1, space="PSUM") as ps_y,
            tc.tile_pool(name="dram", bufs=1, space="DRAM") as dram,
        ):
            partial = dram.tile([T, H], BF16)  # collective input bounce
            if combine == "rs":
                reduced = dram.tile([TS, H], BF16)
            else:
                reduced = dram.tile([T, H], BF16)

            def body(_iv=None):
                # ---- activations + router/gather constants (fresh each iter)
                z = zpool.tile([128, HK, T], F32, tag="z")
                xn_sb = zpool.tile([128, TK, H], BF16, tag="xn")
                g_sb = zpool.tile([128, HK, E], F32, tag="g")
                cum_sb = zpool.tile([128, 2 * 128], F32, tag="cum")
                iota_sb = zpool.tile([128, C], F32, tag="iota")
                id_sb = zpool.tile([128, 128], BF16, tag="id")
                if ablate != "mlp":
                    nc.gpsimd.dma_start(z[:], xT_v)
                    nc.gpsimd.dma_start(xn_sb[:], xn_v)
                    nc.gpsimd.dma_start(g_sb[:], gate_v)
                    nc.gpsimd.dma_start(cum_sb[:], cum.ap())
                    nc.gpsimd.dma_start(iota_sb[:], iota.ap())
                    nc.gpsimd.dma_start(id_sb[:], ident.ap())

                if ablate == "mlp":
                    # isolate weights DMA + MLP + tail: fake the routing
                    id_sb2 = id_sb
                    z_g = zpool.tile([128, HK, C], BF16, tag="zg")
                    st_sb = gpool.tile([128, TK, 128], BF16, tag="st")
                    cg_sb = small.tile([128, 1], F32, tag="cg")
                    nc.gpsimd.dma_start(id_sb2[:], ident.ap())
                    nc.vector.memset(z_g[:], 0.0)
                    nc.vector.memset(st_sb[:], 0.0)
                    nc.vector.memset(cg_sb[:], 0.0)
                    run_mlp(z_g, st_sb, cg_sb, id_sb2, None)
                    return

                # ---- router: logits -> softmax -> top-2 renormalized weight
                # for THIS core's expert (gate column 0). cb[t] in [128,1] is
                # the combine weight, 0 when the token skips this expert;
                # sel[t] is the 0/1 selection mask.
                combs, masks = [], []
                for t in range(TK):
                    ps_r = (ps_a if t == 0 else ps_h1).tile(
                        [128, E], F32, tag="ra" if t == 0 else "h1"
                    )
                    for hk in range(HK):
                        nc.tensor.matmul(
                            ps_r[:],
                            z[:, hk, ts(t, 128)],
                            g_sb[:, hk, :],
                            start=(hk == 0),
                            stop=(hk == HK - 1),
                        )
                    neg_mx = small.tile([128, 1], F32, tag="neg_mx")
                    nc.vector.tensor_reduce(
                        neg_mx[:], ps_r[:], AX.X, ALU.max, negate=True
                    )
                    ex = small.tile([128, E], F32, tag="ex")
                    ssum = small.tile([128, 1], F32, tag="ssum")
                    nc.scalar.activation(
                        ex[:], ps_r[:], AF.Exp, bias=neg_mx[:], accum_out=ssum[:]
                    )
                    srec = small.tile([128, 1], F32, tag="srec")
                    nc.vector.reciprocal(srec[:], ssum[:])
                    p = small.tile([128, E], F32, tag="p")
                    nc.vector.tensor_scalar_mul(p[:], ex[:], srec[:])
                    m1 = small.tile([128, 1], F32, tag="m1")
                    nc.vector.tensor_reduce(m1[:], p[:], AX.X, ALU.max)
                    # knock out the top-1 entry; the max of the rest is top-2
                    pm = small.tile([128, E], F32, tag="pm")
                    nc.vector.tensor_single_scalar(pm[:], p[:], m1[:], ALU.is_equal)
                    p2 = small.tile([128, E], F32, tag="p2")
                    nc.vector.scalar_tensor_tensor(
                        p2[:], pm[:], -2.0, p[:], ALU.mult, ALU.add
                    )
                    m2 = small.tile([128, 1], F32, tag="m2")
                    nc.vector.tensor_reduce(m2[:], p2[:], AX.X, ALU.max)
                    denom = small.tile([128, 1], F32, tag="denom")
                    nc.vector.tensor_add(denom[:], m1[:], m2[:])
                    drec = small.tile([128, 1], F32, tag="drec")
                    nc.vector.reciprocal(drec[:], denom[:])
                    sel = small.tile([128, 1], F32, tag="sel")
                    nc.vector.tensor_single_scalar(
                        sel[:], p[:, 0:1], m2[:], ALU.is_ge
                    )
                    wn = small.tile([128, 1], F32, tag="wn")
                    nc.vector.tensor_scalar_mul(wn[:], p[:, 0:1], drec[:])
                    cb = small.tile([128, 1], F32, tag="cb")
                    nc.vector.tensor_mul(cb[:], wn[:], sel[:])
                    combs.append(cb)
                    masks.append(sel)
                    if debug_comb:
                        nc.sync.dma_start(combdbg[ts(t, 128), :], cb[:])

                # ---- token compaction: exclusive cumsum of the mask gives
                # each selected token its capacity slot; S (and its PE
                # transpose St) are the one-hot gather/scatter matrices.
                s_bf, s_f32 = [], []
                st_sb = gpool.tile([128, TK, 128], BF16, tag="st")
                for t in range(TK):
                    pos_ps = ps_a.tile([128, 1], F32, tag="ra")
                    if t == 0:
                        nc.tensor.matmul(
                            pos_ps[:], cum_sb[:, 0:128], masks[0][:],
                            start=True, stop=True,
                        )
                    else:
                        nc.tensor.matmul(
                            pos_ps[:], cum_sb[:, 128:256], masks[0][:],
                            start=True, stop=False,
                        )
                        nc.tensor.matmul(
                            pos_ps[:], cum_sb[:, 0:128], masks[1][:],
                            start=False, stop=True,
                        )
                    pos_sb = small.tile([128, 1], F32, tag="pos")
                    nc.scalar.copy(pos_sb[:], pos_ps[:])
                    if debug_comb:
                        nc.sync.dma_start(posdbg[ts(t, 128), :], pos_sb[:])
                    s_eq = gpool.tile([128, C], F32, tag="s_eq")
                    nc.vector.tensor_single_scalar(
                        s_eq[:], iota_sb[:], pos_sb[:], ALU.is_equal
                    )
                    s_m = gpool.tile([128, C], F32, tag="s_m")
                    nc.vector.tensor_scalar_mul(s_m[:], s_eq[:], masks[t][:])
                    s_b = gpool.tile([128, C], BF16, tag="s_b")
                    nc.vector.tensor_copy(s_b[:], s_m[:])
                    s_f32.append(s_m)
                    s_bf.append(s_b)

                cg_sb = small.tile([128, 1], F32, tag="cg")

                def st_and_cg():
                    # St/comb_g are only needed by the tail scatter: compute
                    # them off the critical head path, during MLP group 0.
                    for t in range(TK):
                        tr_ps = ps_t.tile([128, 128], BF16, tag="tr")
                        nc.tensor.transpose(tr_ps[:], s_bf[t][:], id_sb[:])
                        nc.scalar.copy(st_sb[:, t, :], tr_ps[:])
                    # per-slot combine weight, exact fp32: comb_g = S^T @ comb
                    cg_ps = ps_a.tile([128, 1], F32, tag="ra")
                    for t in range(TK):
                        nc.tensor.matmul(
                            cg_ps[:], s_f32[t][:], combs[t][:],
                            start=(t == 0), stop=(t == TK - 1),
                        )
                    nc.scalar.copy(cg_sb[:], cg_ps[:])

                # gathered activations, transposed layout: xg[h, c]
                z_g = zpool.tile([128, HK, C], BF16, tag="zg")
                for hk in range(HK):
                    xg_ps = ps_h1.tile([128, C], F32, tag="h1")
                    for t in range(TK):
                        nc.tensor.matmul(
                            xg_ps[:],
                            xn_sb[:, t, ts(hk, 128)],
                            s_bf[t][:],
                            start=(t == 0),
                            stop=(t == TK - 1),
                        )
                    nc.scalar.copy(z_g[:, hk, :], xg_ps[:])

                run_mlp(z_g, st_sb, cg_sb, id_sb, st_and_cg)

            def run_mlp(z_g, st_sb, cg_sb, id_sb, st_and_cg=None):
                # ---- expert MLP on the C slots, grouped weight streaming.
                # z_g is the STATIONARY operand and the w1/w3 group columns
                # the moving one: 8 matmuls of 512 moving columns per group
                # instead of 32 of 128, slashing PE instruction count and
                # weight-load rows. The SwiGLU product comes out slot-major
                # [c, i]; PE-transpose it back to i-major 128-blocks for the
                # W2 accumulation chain.
                yg_ps = ps_y.tile([128, H], F32, tag="yg")
                hm_ci = [None] * GROUPS
                w2_sbs = {}

                def stage_w2(m):
                    s = W2_STAGE_OF[m]
                    if m != W2_START[s]:
                        return
                    nch = W2_STAGES[s]
                    w2_sbs[s] = w2p.tile(
                        [128, nch, H], BF16, tag="w2", name=f"w2sb{s}"
                    )
                    nc.sync.dma_start(
                        w2_sbs[s][:], w2_v[:, bass_ds(W2_START[s], nch), :]
                    )

                def tr_and_w2(g):
                    # transpose group g's SwiGLU product to i-major, then run
                    # its W2 accumulation chains. All transposes are issued
                    # first so PE never stalls on the psum->sbuf copy of the
                    # transpose it is about to consume.
                    hmts = []
                    for j in range(MPG):
                        tr_ps = ps_t.tile([128, 128], BF16, tag="tr")
                        nc.tensor.transpose(
                            tr_ps[:], hm_ci[g][:, ts(j, 128)], id_sb[:]
                        )
                        hmt = hmtp.tile([128, 128], BF16, tag="hmt")
                        nc.scalar.copy(hmt[:], tr_ps[:])
                        hmts.append(hmt)
                    for j in range(MPG):
                        m = g * MPG + j
                        s = W2_STAGE_OF[m]
                        off = m - W2_START[s]
                        for n in range(NH):
                            nc.tensor.matmul(
                                yg_ps[:, ts(n, 512)],
                                hmts[j][:],
                                w2_sbs[s][:, off, ts(n, 512)],
                                start=(m == 0),
                                stop=(m == MK - 1),
                            )

                for g in range(GROUPS):
                    # first W2 stage goes ahead of w1/w3 in the DMA FIFO so
                    # the first W2 matmul chain never head-of-line-blocks PE
                    for j in range(MPG):
                        stage_w2(g * MPG + j)
                    w1_sb = w1p.tile([128, HK, IG], BF16, tag="w1")
                    w3_sb = w3p.tile([128, HK, IG], BF16, tag="w3")
                    nc.sync.dma_start(w1_sb[:], w1_v[:, g, :])
                    nc.sync.dma_start(w3_sb[:], w3_v[:, g, :])
                    h1m = ps_h1.tile([128, IG], F32, tag="h1")
                    h3m = ps_h3.tile([128, IG], F32, tag="h3")
                    # interleave the h1/h3 chains so consecutive matmuls share
                    # the same stationary tile z_g[:, hk, :]
                    for hk in range(HK):
                        nc.tensor.matmul(
                            h1m[:],
                            z_g[:, hk, :],
                            w1_sb[:, hk, :],
                            start=(hk == 0),
                            stop=(hk == HK - 1),
                        )
                        nc.tensor.matmul(
                            h3m[:],
                            z_g[:, hk, :],
                            w3_sb[:, hk, :],
                            start=(hk == 0),
                            stop=(hk == HK - 1),
                        )
                    if g == 0 and st_and_cg is not None:
                        st_and_cg()
                    h1s = hpool.tile([128, IG], F32, tag="h1s")
                    if silu_native:
                        nc.scalar.activation(h1s[:], h1m[:], AF.Silu)
                    else:
                        sg = hpool.tile([128, IG], F32, tag="sg")
                        nc.scalar.activation(sg[:], h1m[:], AF.Sigmoid)
                        nc.vector.tensor_mul(h1s[:], sg[:], h1m[:])
                    hm = hpool.tile([128, IG], BF16, tag="hmci")
                    nc.vector.tensor_mul(hm[:], h1s[:], h3m[:])
                    hm_ci[g] = hm
                    # transposes + W2 for the previous group: gives ACT/DVE a
                    # full group of slack to produce hm before PE needs it.
                    if g >= 1:
                        tr_and_w2(g - 1)
                tr_and_w2(GROUPS - 1)

                # ---- scale by combine weight, scatter to token order,
                # store. Done in H-halves x token-chunks so the four scatter
                # quarter-chains pipeline across PE/ACT/DMA.
                for n in range(NH):
                    ygh = outsb.tile([128, 512], BF16, tag=f"ygs{n}")
                    nc.vector.tensor_scalar_mul(
                        ygh[:], yg_ps[:, ts(n, 512)], cg_sb[:]
                    )
                    for t in range(TK):
                        scq = (ps_h1 if t == 0 else ps_h3).tile(
                            [128, 512], F32, tag="h1" if t == 0 else "h3"
                        )
                        nc.tensor.matmul(
                            scq[:], st_sb[:, t, :], ygh[:],
                            start=True, stop=True,
                        )
                        o_sb = outsb.tile([128, 512], BF16, tag=f"o{t}{n}")
                        nc.scalar.copy(o_sb[:], scq[:])
                        nc.gpsimd.dma_start(
                            partial[ts(t, 128), ts(n, 512)], o_sb[:]
                        )

            if iters == 1:
                body()
            else:
                with tc.For_i(
                    0, iters, 1, hint_engines=(mybir.EngineType.PE,)
                ) as iv:
                    body(iv)

            if with_collective:
                nc.gpsimd.collective_compute(
                    "ReduceScatter" if combine == "rs" else "AllReduce",
                    ALU.add,
                    replica_groups=[list(range(n_cores))],
                    ins=[partial[:].opt()],
                    outs=[reduced[:].opt()],
                )
                nc.sync.dma_start(out[:], reduced[:])
            else:
                nc.sync.dma_start(out[:], partial[:])

    nc.compile()
    return nc


_CACHE = {}


def _built(key):
    if key not in _CACHE:
        _CACHE[key] = build_nc(*key)
    return _CACHE[key]


def _wlayout13(w):
    # [H, I] -> [hi=128, g, ho*ig] so each group DMA is one 8KB
    # descriptor per partition: out[hi, g, ho*IG+ig] = w[ho*128+hi, g*IG+ig]
    return np.ascontiguousarray(
        w.reshape(HK, 128, GROUPS, IG)
        .transpose(1, 2, 0, 3)
        .reshape(128, GROUPS, HK * IG)
        .astype(BF16NP)
    )


def _wlayout2(w):
    # [I, H] -> [ki=128, m, h]: out[ki, m, h] = w[m*128+ki, h]
    return np.ascontiguousarray(
        w.reshape(MK, 128, H).transpose(1, 0, 2).astype(BF16NP)
    )


def make_in_maps(hidden_states, gate_w, w1s, w2s, w3s, n_cores=N_CORES):
    x = np.asarray(hidden_states, dtype=np.float32)
    gate_w = np.asarray(gate_w, dtype=np.float32)
    w1s = np.asarray(w1s, dtype=np.float32)
    w2s = np.asarray(w2s, dtype=np.float32)
    w3s = np.asarray(w3s, dtype=np.float32)

    # x^T [H, T] -> [hi=128, ho, t] (fp32, feeds the exact router matmul)
    xT = np.ascontiguousarray(
        x.T.reshape(HK, 128, T).transpose(1, 0, 2).astype(np.float32)
    )
    # x [T, H] -> [ti=128, to, h] (bf16, stationary operand of the gather)
    xn = np.ascontiguousarray(
        x.reshape(TK, 128, H).transpose(1, 0, 2).astype(BF16NP)
    )
    # cumsum matrix: [:, 0:128][p, t] = (p < t), [:, 128:256] all-ones
    cum = np.zeros((128, 256), np.float32)
    cum[:, 0:128] = np.tril(np.ones((128, 128), np.float32), k=-1).T
    cum[:, 128:256] = 1.0
    iota = np.tile(np.arange(C, dtype=np.float32), (128, 1))
    ident = np.eye(128, dtype=BF16NP)

    in_maps = []
    for c in range(n_cores):
        g = np.roll(gate_w, -c, axis=1)  # column 0 = this core's expert
        in_maps.append(
            {
                "xT": xT,
                "xn": xn,
                "gate": np.ascontiguousarray(
                    g.reshape(HK, 128, E).transpose(1, 0, 2)
                ),
                "cum": cum,
                "iota": iota,
                "ident": ident,
                "w1": _wlayout13(w1s[c]),
                "w3": _wlayout13(w3s[c]),
                "w2": _wlayout2(w2s[c]),
            }
        )
    return in_maps


def kernel(hidden_states, gate_w, w1s, w2s, w3s):
    in_maps = make_in_maps(hidden_states, gate_w, w1s, w2s, w3s)
    nc = _built((1, N_CORES, True))
    res = run_bass_kernel_spmd(nc, in_maps, core_ids=list(range(N_CORES)))
    # ReduceScatter leaves token shard c on core c; concatenate the shards.
    return np.concatenate(
        [np.asarray(res.results[c]["out"]) for c in range(N_CORES)], axis=0
    ).astype(np.float32)
